# revision 6
# baseline (speedup 1.0000x reference)
"""Trainium2 Bass kernel for nn_Dota2Eq3Embed (2-tower equivariant set-net).

Math restructure (vs the reference einsum chain):
  e[n,i,:] = relu(embed[x[n,t,i]])                 (5 team members, d=64)
  t_{ijk}  = e_i * e_j * e_k   (elementwise over d)
  The 8 equivariant pooled ops all factor through S = sum_i e_i:
    op0 = t_{ijk}, op1 = S*e_j*e_k, op2 = S*e_i*e_k, op3 = S*e_i*e_j,
    op4 = S^2*e_k, op5 = S^2*e_j, op6 = S^2*e_i, op7 = S^3
  eq3 output at (i,j,k):  out[s,ijk] = sum_g sum_d C_g[d,s] * F_g[d, m_g(ijk)]
  -> 8 PSUM-accumulated matmuls per sample with FIXED stationaries C_g and
     moving operands that are broadcast access-patterns over tiny feature
     tensors (u = pair products, SU = S*u, S2e = S^2*e, S3).
  The mean over (i,j,k) commutes with the second-layer contraction:
    t1e = relu( (1/125 * sum_ijk relu(out[s,ijk] + bias)) @ wout + bout )
  so the big (N,5,5,5,128) tensors of the reference are never materialized.

Sharding: pure data parallelism over the batch (2048 -> 8 x 256). The two
towers of one sample ride in SBUF partition halves 0-63 / 64-127 (d=64), so
every DVE op runs at full 128-partition width and the per-tower coefficient
stacks live in the matching partition halves for row-tiled matmuls.
"""

import os
import sys
import dataclasses

import numpy as np

try:
    import concourse.bass as bass  # noqa: F401
except Exception:  # pragma: no cover - fresh grading container
    for _p in ("/opt/trn_rl_repo", "/root/.axon_site/_ro/trn_rl_repo"):
        if os.path.isdir(_p) and _p not in sys.path:
            sys.path.insert(0, _p)
    import concourse.bass as bass

import concourse.mybir as mybir
from concourse import bacc, tile
from concourse.bass_utils import run_bass_kernel_spmd

F32 = mybir.dt.float32
I32 = mybir.dt.int32
ALU = mybir.AluOpType
ACTF = mybir.ActivationFunctionType
AXIS = mybir.AxisListType

N_CORES = 8
BATCH = 2048
N_LOC = BATCH // N_CORES          # 256 samples per core
TEAM = 5
D = 64                            # embed dim
HID = 128
OUT_DIM = 128
NEMBED = 128
NPAIR = N_LOC                     # one pair = (tower1, tower2) of one sample
NB = 64                           # pairs per feature block
NBLK = NPAIR // NB                # 4 blocks
WPAIRS = 8                        # pairs per psum window (16 tiles x 125 = 4 banks)
NWIN_PER_BLK = NB // WPAIRS       # 8 windows per block


def _cap(tile_handle, plo, phi, coloff, pairs):
    """AP over tile partitions [plo:phi) with custom free-dim [step,count] list."""
    sl = tile_handle[plo:phi, 0:1]
    return dataclasses.replace(
        sl, offset=sl.offset + coloff, ap=[list(sl.ap[0])] + [list(p) for p in pairs]
    )


def build_nc():
    nc = bacc.Bacc(None, target_bir_lowering=False)

    x_d = nc.dram_tensor("x", [1, N_LOC * 2 * TEAM], I32, kind="ExternalInput")
    emb_d = nc.dram_tensor("embed", [NEMBED, D], F32, kind="ExternalInput")
    cstk_d = nc.dram_tensor("cstk", [128, 8 * HID], F32, kind="ExternalInput")
    bias1_d = nc.dram_tensor("bias1", [HID, 1], F32, kind="ExternalInput")
    bias2_d = nc.dram_tensor("bias2", [HID, 1], F32, kind="ExternalInput")
    wout1_d = nc.dram_tensor("wout1s", [HID, OUT_DIM], F32, kind="ExternalInput")
    wout2_d = nc.dram_tensor("wout2s", [HID, OUT_DIM], F32, kind="ExternalInput")
    bout1_d = nc.dram_tensor("bout1", [OUT_DIM, 1], F32, kind="ExternalInput")
    bout2_d = nc.dram_tensor("bout2", [OUT_DIM, 1], F32, kind="ExternalInput")
    fcwhi_d = nc.dram_tensor("fcwhi", [OUT_DIM, 2], F32, kind="ExternalInput")
    fcwlo_d = nc.dram_tensor("fcwlo", [OUT_DIM, 2], F32, kind="ExternalInput")
    fcb_d = nc.dram_tensor("fcb", [1, 2], F32, kind="ExternalInput")
    ones_d = nc.dram_tensor("ones", [1, 128], F32, kind="ExternalInput")
    iota_d = nc.dram_tensor("iota", [128, 1], F32, kind="ExternalInput")
    out_d = nc.dram_tensor("out", [N_LOC, 2], F32, kind="ExternalOutput")

    NX = N_LOC * 2 * TEAM  # 2560 one-hot columns, col = (n*2+t)*5 + i

    with tile.TileContext(nc) as tc:
        with (
            tc.tile_pool(name="const", bufs=1) as cp,
            tc.tile_pool(name="feat", bufs=2) as fp,
            tc.tile_pool(name="relu", bufs=3) as rp,
            tc.tile_pool(name="ps", bufs=2, space="PSUM") as pp,
        ):
            # ---- params -> SBUF ----
            emb_raw = cp.tile([NEMBED, D], F32)
            nc.sync.dma_start(emb_raw[:, :], emb_d[:, :])
            rel_emb = cp.tile([NEMBED, D], F32)
            nc.vector.tensor_scalar_max(rel_emb[:, :], emb_raw[:, :], 0.0)

            cstk = cp.tile([128, 8 * HID], F32)
            nc.sync.dma_start(cstk[:, :], cstk_d[:, :])
            wout1s = cp.tile([HID, OUT_DIM], F32)
            nc.sync.dma_start(wout1s[:, :], wout1_d[:, :])
            wout2s = cp.tile([HID, OUT_DIM], F32)
            nc.sync.dma_start(wout2s[:, :], wout2_d[:, :])
            bias1 = cp.tile([HID, 1], F32)
            nc.sync.dma_start(bias1[:, :], bias1_d[:, :])
            bias2 = cp.tile([HID, 1], F32)
            nc.sync.dma_start(bias2[:, :], bias2_d[:, :])
            bout1 = cp.tile([OUT_DIM, 1], F32)
            nc.sync.dma_start(bout1[:, :], bout1_d[:, :])
            bout2 = cp.tile([OUT_DIM, 1], F32)
            nc.sync.dma_start(bout2[:, :], bout2_d[:, :])
            fcwhi = cp.tile([OUT_DIM, 2], F32)
            nc.sync.dma_start(fcwhi[:, :], fcwhi_d[:, :])
            fcwlo = cp.tile([OUT_DIM, 2], F32)
            nc.sync.dma_start(fcwlo[:, :], fcwlo_d[:, :])
            fcb = cp.tile([1, 2], F32)
            nc.sync.dma_start(fcb[:, :], fcb_d[:, :])
            ones = cp.tile([1, 128], F32)
            nc.sync.dma_start(ones[:, :], ones_d[:, :])
            iota = cp.tile([128, 1], F32)
            nc.sync.dma_start(iota[:, :], iota_d[:, :])

            # ---- one-hot of x ----
            xsb = cp.tile([1, NX], I32)
            nc.sync.dma_start(xsb[:, :], x_d[:, :])
            xf = cp.tile([1, NX], F32)
            nc.vector.tensor_copy(xf[:, :], xsb[:, :])

            onehot = cp.tile([128, NX], F32)
            for c in range(NX // 512):
                pidx = pp.tile([128, 512], F32, tag="ps")
                nc.tensor.matmul(
                    pidx[:, :], ones[:, :], xf[:, c * 512:(c + 1) * 512],
                    start=True, stop=True,
                )
                nc.vector.tensor_scalar(
                    onehot[:, c * 512:(c + 1) * 512], pidx[:, :],
                    iota[:, 0:1], None, op0=ALU.is_equal,
                )

            # ---- gather: e_sb[0:64] = even st (tower1), [64:128] = odd (tower2)
            # e_sb col = pair*5 + i, value = relu(embed[x])[d]
            e_sb = cp.tile([128, NPAIR * TEAM], F32)
            GCH = 64  # pairs per gather chunk -> 320 psum cols
            for c in range(NPAIR // GCH):
                pg = pp.tile([128, GCH * TEAM], F32, tag="ps")
                for h in range(2):
                    rhs = _cap(onehot, 0, 128, c * GCH * 10 + h * TEAM,
                               [[10, GCH], [1, TEAM]])
                    nc.tensor.matmul(
                        pg[h * 64:(h + 1) * 64, :], rel_emb[:, :], rhs,
                        start=True, stop=True, tile_position=(0, h * 64),
                    )
                nc.scalar.copy(
                    e_sb[:, c * GCH * TEAM:(c + 1) * GCH * TEAM], pg[:, :])

            hbarA = cp.tile([HID, NPAIR], F32)
            hbarB = cp.tile([HID, NPAIR], F32)

            for b in range(NBLK):
                ecol = b * NB * TEAM
                # ---- features for NB pairs (both towers via partition halves)
                u = fp.tile([128, NB * 25], F32)
                nc.vector.tensor_mul(
                    u[:, :],
                    _cap(e_sb, 0, 128, ecol, [[5, NB], [1, 5], [0, 5]]),
                    _cap(e_sb, 0, 128, ecol, [[5, NB], [0, 5], [1, 5]]),
                )
                t3 = fp.tile([128, NB * 125], F32)
                nc.vector.tensor_mul(
                    t3[:, :],
                    _cap(e_sb, 0, 128, ecol, [[5, NB], [1, 5], [0, 25]]),
                    _cap(u, 0, 128, 0, [[25, NB], [0, 5], [1, 25]]),
                )
                S = fp.tile([128, NB], F32)
                nc.vector.tensor_reduce(
                    S[:, :],
                    _cap(e_sb, 0, 128, ecol, [[5, NB], [1, 5]]),
                    axis=AXIS.X, op=ALU.add,
                )
                SU = fp.tile([128, NB * 25], F32)
                nc.vector.tensor_mul(
                    SU[:, :], u[:, :],
                    _cap(S, 0, 128, 0, [[1, NB], [0, 25]]),
                )
                S2 = fp.tile([128, NB], F32)
                nc.vector.tensor_mul(S2[:, :], S[:, :], S[:, :])
                S2e = fp.tile([128, NB * TEAM], F32)
                nc.vector.tensor_mul(
                    S2e[:, :],
                    _cap(e_sb, 0, 128, ecol, [[5, NB], [1, 5]]),
                    _cap(S2, 0, 128, 0, [[1, NB], [0, 5]]),
                )
                S3 = fp.tile([128, NB], F32)
                nc.vector.tensor_mul(S3[:, :], S2[:, :], S[:, :])

                # rhs APs covering a whole bank-group of 4 pairs starting at pb0
                def grp_rhs4(g, h, pb0):
                    lo, hi = h * 64, (h + 1) * 64
                    if g == 0:
                        return _cap(t3, lo, hi, pb0 * 125, [[1, 500]])
                    if g == 1:
                        return _cap(SU, lo, hi, pb0 * 25, [[25, 4], [0, 5], [1, 25]])
                    if g == 2:
                        return _cap(SU, lo, hi, pb0 * 25, [[5, 20], [0, 5], [1, 5]])
                    if g == 3:
                        return _cap(SU, lo, hi, pb0 * 25, [[5, 20], [1, 5], [0, 5]])
                    if g == 4:
                        return _cap(S2e, lo, hi, pb0 * 5, [[5, 4], [0, 25], [1, 5]])
                    if g == 6:
                        return _cap(S2e, lo, hi, pb0 * 5, [[5, 4], [1, 5], [0, 25]])
                    return _cap(S3, lo, hi, pb0, [[1, 4], [0, 125]])  # g == 7

                for wb in range(NWIN_PER_BLK):
                    # pa: 4 banks of 512 f32; bank holds 4 pairs x 125 cols.
                    # banks 0-1 = A half (8 pairs), banks 2-3 = B half.
                    pa = pp.tile([128, 2048], F32, tag="ps")
                    for g in range(8):
                        for h in range(2):
                            lhs = cstk[h * 64:(h + 1) * 64, g * HID:(g + 1) * HID]
                            tp = (h * 64, 0)
                            for kb in range(2):  # bank within half
                                bank = h * 2 + kb
                                pb0 = wb * WPAIRS + kb * 4
                                bsl = pa[:, bank * 512:bank * 512 + 500]
                                if g == 5:
                                    # S^2*e_j broadcast needs 4 free dims; per pair
                                    for q in range(4):
                                        nc.tensor.matmul(
                                            pa[:, bank * 512 + q * 125:
                                               bank * 512 + (q + 1) * 125],
                                            lhs,
                                            _cap(S2e, h * 64, (h + 1) * 64,
                                                 (pb0 + q) * 5,
                                                 [[0, 5], [1, 5], [0, 5]]),
                                            start=False, stop=False,
                                            tile_position=tp,
                                        )
                                else:
                                    nc.tensor.matmul(
                                        bsl, lhs, grp_rhs4(g, h, pb0),
                                        start=(g == 0), stop=(g == 7),
                                        tile_position=tp,
                                    )
                    ra = rp.tile([128, 2 * WPAIRS * 125], F32, tag="ra")
                    nc.scalar.activation(
                        _cap(ra, 0, 128, 0, [[500, 2], [125, 4], [1, 125]]),
                        _cap(pa, 0, 128, 0, [[512, 2], [125, 4], [1, 125]]),
                        ACTF.Relu, bias=bias1[:, 0:1])
                    nc.scalar.activation(
                        _cap(ra, 0, 128, 1000, [[500, 2], [125, 4], [1, 125]]),
                        _cap(pa, 0, 128, 1024, [[512, 2], [125, 4], [1, 125]]),
                        ACTF.Relu, bias=bias2[:, 0:1])
                    pcol = b * NB + wb * WPAIRS
                    nc.vector.tensor_reduce(
                        hbarA[:, pcol:pcol + WPAIRS],
                        _cap(ra, 0, 128, 0, [[125, WPAIRS], [1, 125]]),
                        axis=AXIS.X, op=ALU.add,
                    )
                    nc.vector.tensor_reduce(
                        hbarB[:, pcol:pcol + WPAIRS],
                        _cap(ra, 0, 128, 1000, [[125, WPAIRS], [1, 125]]),
                        axis=AXIS.X, op=ALU.add,
                    )

            # ---- layer 2: t{1,2}e = relu(hbar @ (wout/125) + bout) ----
            # separate banks: a start=True zeroes its whole 2KB psum bank
            p2 = pp.tile([128, 1024], F32, tag="ps")
            nc.tensor.matmul(p2[:, 0:NPAIR], wout1s[:, :], hbarA[:, :],
                             start=True, stop=True)
            nc.tensor.matmul(p2[:, 512:512 + NPAIR], wout2s[:, :], hbarB[:, :],
                             start=True, stop=True)
            z1 = cp.tile([OUT_DIM, NPAIR], F32)
            z2 = cp.tile([OUT_DIM, NPAIR], F32)
            nc.scalar.activation(z1[:, :], p2[:, 0:NPAIR], ACTF.Relu,
                                 bias=bout1[:, 0:1])
            nc.scalar.activation(z2[:, :], p2[:, 512:512 + NPAIR], ACTF.Relu,
                                 bias=bout2[:, 0:1])

            # ---- final fc: out = z @ fcw + fcb ----
            outsb = cp.tile([128, 2 * (N_LOC // 128)], F32)
            pfc = pp.tile([128, 1024], F32, tag="ps")
            for ch in range(N_LOC // 128):
                sl = pfc[:, ch * 512:ch * 512 + 2]
                nc.tensor.matmul(sl, z1[:, ch * 128:(ch + 1) * 128],
                                 fcwhi[:, :], start=True, stop=False)
                nc.tensor.matmul(sl, z2[:, ch * 128:(ch + 1) * 128],
                                 fcwlo[:, :], start=False, stop=False)
                nc.tensor.matmul(sl, ones[:, :], fcb[:, :],
                                 start=False, stop=True)
            nc.vector.tensor_copy(
                _cap(outsb, 0, 128, 0, [[2, 2], [1, 2]]),
                _cap(pfc, 0, 128, 0, [[512, 2], [1, 2]]),
            )
            for ch in range(N_LOC // 128):
                nc.sync.dma_start(out_d[ch * 128:(ch + 1) * 128, :],
                                  outsb[:, ch * 2:(ch + 1) * 2])

    nc.compile()
    return nc


def make_in_maps(inputs):
    x = np.ascontiguousarray(np.asarray(inputs["x"], dtype=np.int32))
    embed = np.asarray(inputs["embed"], dtype=np.float32)
    coefs1 = np.asarray(inputs["coefs1"], dtype=np.float32)
    coefs2 = np.asarray(inputs["coefs2"], dtype=np.float32)

    cstk = np.zeros((128, 8 * HID), dtype=np.float32)
    for g in range(8):
        cstk[0:64, g * HID:(g + 1) * HID] = coefs1[:, :, g]
        cstk[64:128, g * HID:(g + 1) * HID] = coefs2[:, :, g]

    common = {
        "embed": np.ascontiguousarray(embed),
        "cstk": cstk,
        "bias1": np.asarray(inputs["bias1"], np.float32).reshape(HID, 1).copy(),
        "bias2": np.asarray(inputs["bias2"], np.float32).reshape(HID, 1).copy(),
        "wout1s": np.ascontiguousarray(np.asarray(inputs["wout1"], np.float32) / 125.0),
        "wout2s": np.ascontiguousarray(np.asarray(inputs["wout2"], np.float32) / 125.0),
        "bout1": np.asarray(inputs["bout1"], np.float32).reshape(OUT_DIM, 1).copy(),
        "bout2": np.asarray(inputs["bout2"], np.float32).reshape(OUT_DIM, 1).copy(),
        "fcwhi": np.ascontiguousarray(np.asarray(inputs["fcw"], np.float32)[0:OUT_DIM]),
        "fcwlo": np.ascontiguousarray(np.asarray(inputs["fcw"], np.float32)[OUT_DIM:]),
        "fcb": np.asarray(inputs["fcb"], np.float32).reshape(1, 2).copy(),
        "ones": np.ones((1, 128), np.float32),
        "iota": np.arange(128, dtype=np.float32).reshape(128, 1).copy(),
    }
    in_maps = []
    for c in range(N_CORES):
        m = dict(common)
        m["x"] = x[c * N_LOC:(c + 1) * N_LOC].reshape(1, N_LOC * 2 * TEAM).copy()
        in_maps.append(m)
    return in_maps


_NC = None


def kernel(**inputs):
    global _NC
    if _NC is None:
        _NC = build_nc()
    in_maps = make_in_maps(inputs)
    res = run_bass_kernel_spmd(_NC, in_maps, core_ids=list(range(N_CORES)))
    return np.concatenate([r["out"] for r in res.results], axis=0)


if __name__ == "__main__":
    nc = build_nc()
    print("built ok:", len(nc.m.functions[0].allocations), "allocations")


# revision 12
# speedup vs baseline: 2.8718x; 2.8718x over previous
"""Trainium2 Bass kernel for nn_Dota2Eq3Embed (2-tower equivariant set-net).

Math restructure (vs the reference einsum chain):
  e[n,i,:] = relu(embed[x[n,t,i]])                 (5 team members, d=64)
  t_{ijk}  = e_i * e_j * e_k   (elementwise over d)
  The 8 equivariant pooled ops all factor through S = sum_i e_i:
    op0 = t_{ijk}, op1 = S*e_j*e_k, op2 = S*e_i*e_k, op3 = S*e_i*e_j,
    op4 = S^2*e_k, op5 = S^2*e_j, op6 = S^2*e_i, op7 = S^3
  eq3 output at (i,j,k):  out[s,ijk] = sum_g sum_d C_g[d,s] * F_g[d, m_g(ijk)]
  -> 8 PSUM-accumulated matmuls per sample with FIXED stationaries C_g and
     moving operands that are broadcast access-patterns over tiny feature
     tensors (u = pair products, SU = S*u, S2e = S^2*e, S3).
  The mean over (i,j,k) commutes with the second-layer contraction:
    t1e = relu( (1/125 * sum_ijk relu(out[s,ijk] + bias)) @ wout + bout )
  so the big (N,5,5,5,128) tensors of the reference are never materialized.

Sharding: pure data parallelism over the batch (2048 -> 8 x 256). The two
towers of one sample ride in SBUF partition halves 0-63 / 64-127 (d=64), so
every DVE op runs at full 128-partition width and the per-tower coefficient
stacks live in the matching partition halves for row-tiled matmuls.
"""

import os
import sys
import dataclasses

import numpy as np

try:
    import concourse.bass as bass  # noqa: F401
except Exception:  # pragma: no cover - fresh grading container
    for _p in ("/opt/trn_rl_repo", "/root/.axon_site/_ro/trn_rl_repo"):
        if os.path.isdir(_p) and _p not in sys.path:
            sys.path.insert(0, _p)
    import concourse.bass as bass

import concourse.mybir as mybir
from concourse import bacc, tile
from concourse.bass_utils import run_bass_kernel_spmd

F32 = mybir.dt.float32
F32R = mybir.dt.float32r
F16 = mybir.dt.float16
BF16 = mybir.dt.bfloat16
I32 = mybir.dt.int32
ALU = mybir.AluOpType
ACTF = mybir.ActivationFunctionType
AXIS = mybir.AxisListType

N_CORES = 8
BATCH = 2048
N_LOC = BATCH // N_CORES          # 256 samples per core
TEAM = 5
D = 64                            # embed dim
HID = 128
OUT_DIM = 128
NEMBED = 128
NPAIR = N_LOC                     # one pair = (tower1, tower2) of one sample
NB = 64                           # pairs per feature block
NBLK = NPAIR // NB                # 4 blocks
WPAIRS = 8                        # pairs per psum window (16 tiles x 125 = 4 banks)
NWIN_PER_BLK = NB // WPAIRS       # 8 windows per block


def _cap(tile_handle, plo, phi, coloff, pairs):
    """AP over tile partitions [plo:phi) with custom free-dim [step,count] list."""
    sl = tile_handle[plo:phi, 0:1]
    return dataclasses.replace(
        sl, offset=sl.offset + coloff, ap=[list(sl.ap[0])] + [list(p) for p in pairs]
    )


def build_nc():
    nc = bacc.Bacc(None, target_bir_lowering=False)

    x_d = nc.dram_tensor("x", [1, N_LOC * 2 * TEAM], I32, kind="ExternalInput")
    emb_d = nc.dram_tensor("embed", [NEMBED, D], F32, kind="ExternalInput")
    cstk_d = nc.dram_tensor("cstk", [128, 8 * HID], F32, kind="ExternalInput")
    bias1_d = nc.dram_tensor("bias1", [HID, 1], F32, kind="ExternalInput")
    bias2_d = nc.dram_tensor("bias2", [HID, 1], F32, kind="ExternalInput")
    wout1_d = nc.dram_tensor("wout1s", [HID, OUT_DIM], F32, kind="ExternalInput")
    wout2_d = nc.dram_tensor("wout2s", [HID, OUT_DIM], F32, kind="ExternalInput")
    bout1_d = nc.dram_tensor("bout1", [OUT_DIM, 1], F32, kind="ExternalInput")
    bout2_d = nc.dram_tensor("bout2", [OUT_DIM, 1], F32, kind="ExternalInput")
    fcwhi_d = nc.dram_tensor("fcwhi", [OUT_DIM, 2], F32, kind="ExternalInput")
    fcwlo_d = nc.dram_tensor("fcwlo", [OUT_DIM, 2], F32, kind="ExternalInput")
    fcb_d = nc.dram_tensor("fcb", [1, 2], F32, kind="ExternalInput")
    ones_d = nc.dram_tensor("ones", [1, 128], F32, kind="ExternalInput")
    iota_d = nc.dram_tensor("iota", [128, 1], F32, kind="ExternalInput")
    out_d = nc.dram_tensor("out", [N_LOC, 2], F32, kind="ExternalOutput")

    NX = N_LOC * 2 * TEAM  # 2560 one-hot columns, col = (n*2+t)*5 + i

    with tile.TileContext(nc) as tc:
        with (
            nc.allow_low_precision(reason="bf16 feature pipeline, f32 psum accum"),
            tc.tile_pool(name="const", bufs=1) as cp,
            tc.tile_pool(name="feat", bufs=2) as fp,
            tc.tile_pool(name="relu", bufs=3) as rp,
            tc.tile_pool(name="ps", bufs=2, space="PSUM") as pp,
        ):
            # ---- params -> SBUF ----
            emb_raw = cp.tile([NEMBED, D], F32)
            nc.sync.dma_start(emb_raw[:, :], emb_d[:, :])
            rel_emb = cp.tile([NEMBED, D], F32)
            nc.vector.tensor_scalar_max(rel_emb[:, :], emb_raw[:, :], 0.0)

            cstk32 = cp.tile([128, 8 * HID], F32)
            nc.sync.dma_start(cstk32[:, :], cstk_d[:, :])
            cstk = cp.tile([128, 8 * HID], F16)
            nc.vector.tensor_copy(cstk[:, :], cstk32[:, :])
            wout1s = cp.tile([HID, OUT_DIM], F32)
            nc.sync.dma_start(wout1s[:, :], wout1_d[:, :])
            wout2s = cp.tile([HID, OUT_DIM], F32)
            nc.sync.dma_start(wout2s[:, :], wout2_d[:, :])
            bias1 = cp.tile([HID, 1], F32)
            nc.sync.dma_start(bias1[:, :], bias1_d[:, :])
            bias2 = cp.tile([HID, 1], F32)
            nc.sync.dma_start(bias2[:, :], bias2_d[:, :])
            bout1 = cp.tile([OUT_DIM, 1], F32)
            nc.sync.dma_start(bout1[:, :], bout1_d[:, :])
            bout2 = cp.tile([OUT_DIM, 1], F32)
            nc.sync.dma_start(bout2[:, :], bout2_d[:, :])
            fcwhi = cp.tile([OUT_DIM, 2], F32)
            nc.sync.dma_start(fcwhi[:, :], fcwhi_d[:, :])
            fcwlo = cp.tile([OUT_DIM, 2], F32)
            nc.sync.dma_start(fcwlo[:, :], fcwlo_d[:, :])
            fcb = cp.tile([1, 2], F32)
            nc.sync.dma_start(fcb[:, :], fcb_d[:, :])
            ones = cp.tile([1, 128], F32)
            nc.sync.dma_start(ones[:, :], ones_d[:, :])
            iota = cp.tile([128, 1], F32)
            nc.sync.dma_start(iota[:, :], iota_d[:, :])

            # ---- one-hot of x ----
            xsb = cp.tile([1, NX], I32)
            nc.sync.dma_start(xsb[:, :], x_d[:, :])
            xf = cp.tile([1, NX], F32)
            nc.vector.tensor_copy(xf[:, :], xsb[:, :])

            onehot = cp.tile([128, NX], F32)
            for c in range(NX // 512):
                pidx = pp.tile([128, 512], F32, tag="ps")
                nc.tensor.matmul(
                    pidx[:, :], ones[:, :], xf[:, c * 512:(c + 1) * 512],
                    start=True, stop=True,
                )
                nc.vector.tensor_scalar(
                    onehot[:, c * 512:(c + 1) * 512], pidx[:, :],
                    iota[:, 0:1], None, op0=ALU.is_equal,
                )

            # ---- gather: e_sb[0:64] = even st (tower1), [64:128] = odd (tower2)
            # e_sb col = pair*5 + i, value = relu(embed[x])[d]
            e_sb = cp.tile([128, NPAIR * TEAM], F32)
            GCH = 64  # pairs per gather chunk -> 320 psum cols
            for c in range(NPAIR // GCH):
                pg = pp.tile([128, GCH * TEAM], F32, tag="ps")
                for h in range(2):
                    rhs = _cap(onehot, 0, 128, c * GCH * 10 + h * TEAM,
                               [[10, GCH], [1, TEAM]])
                    nc.tensor.matmul(
                        pg[h * 64:(h + 1) * 64, :], rel_emb[:, :], rhs,
                        start=True, stop=True, tile_position=(0, h * 64),
                    )
                nc.scalar.copy(
                    e_sb[:, c * GCH * TEAM:(c + 1) * GCH * TEAM], pg[:, :])

            hbarA = cp.tile([HID, NPAIR], F32)
            hbarB = cp.tile([HID, NPAIR], F32)

            for b in range(NBLK):
                ecol = b * NB * TEAM
                # ---- features for NB pairs (both towers via partition halves)
                u = fp.tile([128, NB * 25], F32)
                nc.vector.tensor_mul(
                    u[:, :],
                    _cap(e_sb, 0, 128, ecol, [[5, NB], [1, 5], [0, 5]]),
                    _cap(e_sb, 0, 128, ecol, [[5, NB], [0, 5], [1, 5]]),
                )
                t3 = fp.tile([128, NB * 125], F16)
                nc.vector.tensor_mul(
                    t3[:, :],
                    _cap(e_sb, 0, 128, ecol, [[5, NB], [1, 5], [0, 25]]),
                    _cap(u, 0, 128, 0, [[25, NB], [0, 5], [1, 25]]),
                )
                S = fp.tile([128, NB], F32)
                nc.vector.tensor_reduce(
                    S[:, :],
                    _cap(e_sb, 0, 128, ecol, [[5, NB], [1, 5]]),
                    axis=AXIS.X, op=ALU.add,
                )
                SU = fp.tile([128, NB * 25], F16)
                nc.vector.tensor_mul(
                    SU[:, :], u[:, :],
                    _cap(S, 0, 128, 0, [[1, NB], [0, 25]]),
                )
                S2 = fp.tile([128, NB], F32)
                nc.vector.tensor_mul(S2[:, :], S[:, :], S[:, :])
                S2e = fp.tile([128, NB * TEAM], F16)
                nc.vector.tensor_mul(
                    S2e[:, :],
                    _cap(e_sb, 0, 128, ecol, [[5, NB], [1, 5]]),
                    _cap(S2, 0, 128, 0, [[1, NB], [0, 5]]),
                )
                S3 = fp.tile([128, NB], F16)
                nc.vector.tensor_mul(S3[:, :], S2[:, :], S[:, :])

                # rhs APs covering a whole bank-group of 4 pairs starting at pb0
                def grp_rhs4(g, h, pb0):
                    lo, hi = h * 64, (h + 1) * 64
                    if g == 0:
                        return _cap(t3, lo, hi, pb0 * 125, [[1, 500]])
                    if g == 1:
                        return _cap(SU, lo, hi, pb0 * 25, [[25, 4], [0, 5], [1, 25]])
                    if g == 2:
                        return _cap(SU, lo, hi, pb0 * 25, [[5, 20], [0, 5], [1, 5]])
                    if g == 3:
                        return _cap(SU, lo, hi, pb0 * 25, [[5, 20], [1, 5], [0, 5]])
                    if g == 4:
                        return _cap(S2e, lo, hi, pb0 * 5, [[5, 4], [0, 25], [1, 5]])
                    if g == 6:
                        return _cap(S2e, lo, hi, pb0 * 5, [[5, 4], [1, 5], [0, 25]])
                    return _cap(S3, lo, hi, pb0, [[1, 4], [0, 125]])  # g == 7

                for wb in range(NWIN_PER_BLK):
                    # pa: 4 banks of 512 f32; bank holds 4 pairs x 125 cols.
                    # banks 0-1 = A half (8 pairs), banks 2-3 = B half.
                    pa = pp.tile([128, 2048], F32, tag="ps")
                    for g in range(8):
                        for h in range(2):
                            lhs = cstk[h * 64:(h + 1) * 64, g * HID:(g + 1) * HID]
                            tp = (h * 64, 0)
                            for kb in range(2):  # bank within half
                                bank = h * 2 + kb
                                pb0 = wb * WPAIRS + kb * 4
                                bsl = pa[:, bank * 512:bank * 512 + 500]
                                if g == 5:
                                    # bf16 per-pair: walrus caps matmul APs
                                    # at 3 free dims, so the pair-batched
                                    # j-broadcast AP is not expressible
                                    for q in range(4):
                                        nc.tensor.matmul(
                                            pa[:, bank * 512 + q * 125:
                                               bank * 512 + (q + 1) * 125],
                                            lhs,
                                            _cap(S2e, h * 64, (h + 1) * 64,
                                                 (pb0 + q) * 5,
                                                 [[0, 5], [1, 5], [0, 5]]),
                                            start=False, stop=False,
                                            tile_position=tp,
                                        )
                                else:
                                    nc.tensor.matmul(
                                        bsl, lhs, grp_rhs4(g, h, pb0),
                                        start=(g == 0), stop=(g == 7),
                                        tile_position=tp,
                                    )
                    ra = rp.tile([128, 2 * WPAIRS * 125], F32, tag="ra")
                    nc.scalar.activation(
                        _cap(ra, 0, 128, 0, [[500, 2], [125, 4], [1, 125]]),
                        _cap(pa, 0, 128, 0, [[512, 2], [125, 4], [1, 125]]),
                        ACTF.Relu, bias=bias1[:, 0:1])
                    nc.scalar.activation(
                        _cap(ra, 0, 128, 1000, [[500, 2], [125, 4], [1, 125]]),
                        _cap(pa, 0, 128, 1024, [[512, 2], [125, 4], [1, 125]]),
                        ACTF.Relu, bias=bias2[:, 0:1])
                    pcol = b * NB + wb * WPAIRS
                    nc.vector.tensor_reduce(
                        hbarA[:, pcol:pcol + WPAIRS],
                        _cap(ra, 0, 128, 0, [[125, WPAIRS], [1, 125]]),
                        axis=AXIS.X, op=ALU.add,
                    )
                    nc.vector.tensor_reduce(
                        hbarB[:, pcol:pcol + WPAIRS],
                        _cap(ra, 0, 128, 1000, [[125, WPAIRS], [1, 125]]),
                        axis=AXIS.X, op=ALU.add,
                    )

            # ---- layer 2: t{1,2}e = relu(hbar @ (wout/125) + bout) ----
            # separate banks: a start=True zeroes its whole 2KB psum bank
            p2 = pp.tile([128, 1024], F32, tag="ps")
            nc.tensor.matmul(p2[:, 0:NPAIR], wout1s[:, :], hbarA[:, :],
                             start=True, stop=True)
            nc.tensor.matmul(p2[:, 512:512 + NPAIR], wout2s[:, :], hbarB[:, :],
                             start=True, stop=True)
            z1 = cp.tile([OUT_DIM, NPAIR], F32)
            z2 = cp.tile([OUT_DIM, NPAIR], F32)
            nc.scalar.activation(z1[:, :], p2[:, 0:NPAIR], ACTF.Relu,
                                 bias=bout1[:, 0:1])
            nc.scalar.activation(z2[:, :], p2[:, 512:512 + NPAIR], ACTF.Relu,
                                 bias=bout2[:, 0:1])

            # ---- final fc: out = z @ fcw + fcb ----
            outsb = cp.tile([128, 2 * (N_LOC // 128)], F32)
            pfc = pp.tile([128, 1024], F32, tag="ps")
            for ch in range(N_LOC // 128):
                sl = pfc[:, ch * 512:ch * 512 + 2]
                nc.tensor.matmul(sl, z1[:, ch * 128:(ch + 1) * 128],
                                 fcwhi[:, :], start=True, stop=False)
                nc.tensor.matmul(sl, z2[:, ch * 128:(ch + 1) * 128],
                                 fcwlo[:, :], start=False, stop=False)
                nc.tensor.matmul(sl, ones[:, :], fcb[:, :],
                                 start=False, stop=True)
            nc.vector.tensor_copy(
                _cap(outsb, 0, 128, 0, [[2, 2], [1, 2]]),
                _cap(pfc, 0, 128, 0, [[512, 2], [1, 2]]),
            )
            for ch in range(N_LOC // 128):
                nc.sync.dma_start(out_d[ch * 128:(ch + 1) * 128, :],
                                  outsb[:, ch * 2:(ch + 1) * 2])

    nc.compile()
    return nc


def make_in_maps(inputs):
    x = np.ascontiguousarray(np.asarray(inputs["x"], dtype=np.int32))
    embed = np.asarray(inputs["embed"], dtype=np.float32)
    coefs1 = np.asarray(inputs["coefs1"], dtype=np.float32)
    coefs2 = np.asarray(inputs["coefs2"], dtype=np.float32)

    cstk = np.zeros((128, 8 * HID), dtype=np.float32)
    for g in range(8):
        cstk[0:64, g * HID:(g + 1) * HID] = coefs1[:, :, g]
        cstk[64:128, g * HID:(g + 1) * HID] = coefs2[:, :, g]

    common = {
        "embed": np.ascontiguousarray(embed),
        "cstk": cstk,
        "bias1": np.asarray(inputs["bias1"], np.float32).reshape(HID, 1).copy(),
        "bias2": np.asarray(inputs["bias2"], np.float32).reshape(HID, 1).copy(),
        "wout1s": np.ascontiguousarray(np.asarray(inputs["wout1"], np.float32) / 125.0),
        "wout2s": np.ascontiguousarray(np.asarray(inputs["wout2"], np.float32) / 125.0),
        "bout1": np.asarray(inputs["bout1"], np.float32).reshape(OUT_DIM, 1).copy(),
        "bout2": np.asarray(inputs["bout2"], np.float32).reshape(OUT_DIM, 1).copy(),
        "fcwhi": np.ascontiguousarray(np.asarray(inputs["fcw"], np.float32)[0:OUT_DIM]),
        "fcwlo": np.ascontiguousarray(np.asarray(inputs["fcw"], np.float32)[OUT_DIM:]),
        "fcb": np.asarray(inputs["fcb"], np.float32).reshape(1, 2).copy(),
        "ones": np.ones((1, 128), np.float32),
        "iota": np.arange(128, dtype=np.float32).reshape(128, 1).copy(),
    }
    in_maps = []
    for c in range(N_CORES):
        m = dict(common)
        m["x"] = x[c * N_LOC:(c + 1) * N_LOC].reshape(1, N_LOC * 2 * TEAM).copy()
        in_maps.append(m)
    return in_maps


_NC = None


def kernel(**inputs):
    global _NC
    if _NC is None:
        _NC = build_nc()
    in_maps = make_in_maps(inputs)
    res = run_bass_kernel_spmd(_NC, in_maps, core_ids=list(range(N_CORES)))
    return np.concatenate([r["out"] for r in res.results], axis=0)


if __name__ == "__main__":
    nc = build_nc()
    print("built ok:", len(nc.m.functions[0].allocations), "allocations")
